# revision 32
# baseline (speedup 1.0000x reference)
"""Trainium2 Bass kernel for nn_NetFV (NetFV pooling head), v2.

Strategy (pure data parallel over 8 cores, 256 batches each):
  - fp8e4m3 x in two streams (vs bf16: halves HBM traffic, faster FWL
    weight loads). Moving matmul operands (waug2, act) stay bf16 - the
    PE supports mixed fp8-stationary x bf16-moving exactly.
      xg: natural, host-slabbed per chunk as [x(60)|1|pad3|x^2(60)|1|pad3]
      = 128B so each fv lhsT is one contiguous FWL-eligible 128-col load
      and emits fv1|asum|fv2|asum in one accumulation group.
      xt: transposed double-stacked [x^T_A;1;x^T_B;1], 128 partitions.
  - granule (= 2 superbatch) processing: one exp/reduce/recip/scale
    chain instance per half, split DVE/Pool so halves run in parallel;
    fv trails logits by DG=4 granules so the serial chain never
    head-blocks the in-order PE queue.
  - finishing constants pre-scaled (2^-8 / 2^-24) to keep norm sums in
    the ACT Ln table's valid range; rsqrt = exp(-0.5*ln(x+eps)) plus one
    explicit table load keeps ACT on a single activation table set
    (Sqrt would cost ~2.7us reloads per use).
  - finishing split into 10 single-engine-layer phases, each consuming
    only prior-iteration results (no intra-iteration cross-engine waits);
    stage copies deferred one iteration for the same reason.
"""

import math
import sys

for _p in ("/opt/trn_rl_repo", "/opt/pypackages"):
    if _p not in sys.path:
        sys.path.append(_p)

import ml_dtypes
import numpy as np

import concourse.bacc as bacc
import concourse.bass as bass
import concourse.mybir as mybir
import concourse.tile as tile
from concourse.bass_utils import run_bass_kernel_spmd

F, M, C, OUT = 60, 600, 8, 18
B = 2048
NCORES = 8
BL = B // NCORES            # 256 batches per core
SB = 8                      # batches per superbatch
NSB = BL // SB              # 32 superbatches
FGB = 64                    # batches per finishing group
NFG = BL // FGB             # 4 finishing groups
SBPF = FGB // SB            # 8 superbatches per finishing group
CH = 5                      # chunks (of 120 rows) per batch
RP = M // CH                # 120 rows per chunk
NK = SB * CH                # 40 slab chunks per superbatch
SR = NK * 128               # 5120 slab bytes per sb per partition
HW2 = 2432                  # padded transposed width (2400 + 32)
NW = 20                     # logit windows per superbatch
NG = FGB * C                # 512 finishing columns
PF = 3                      # granule-pair prefetch depth
SHIP_SQ = True              # True: host ships x^2 in the slab (bigger DMA);
                            # False: ship compact x, strided deposit + on-chip x^2
K_ACT = 17                  # x^2 chunk split: ACT chunks
K_DVE = 16                  # DVE chunks (Pool gets the rest)

BF16 = mybir.dt.bfloat16
F8 = mybir.dt.float8e4
F32 = mybir.dt.float32
MULT = mybir.AluOpType.mult
EPS = 1e-12


def _build_nc():
    nc = bacc.Bacc(
        "TRN2", target_bir_lowering=False, debug=False,
        enable_asserts=False, num_devices=NCORES,
    )
    # One granule = 2 superbatches, one contiguous descriptor per
    # partition, 128 partitions on both streams -> 16 DMA engines.
    xg_w = 2 * SR if SHIP_SQ else 2 * SR // 2
    xg = nc.dram_tensor("xg", [NSB // 2, 128, xg_w], F8,
                        kind="ExternalInput").ap()
    xt = nc.dram_tensor("xt", [NSB // 2, 128, 2 * HW2], F8,
                        kind="ExternalInput").ap()
    waug2_d = nc.dram_tensor("waug2", [128, 2 * C], BF16,
                             kind="ExternalInput").ap()
    cst_d = nc.dram_tensor("cst", [128, 3 * C], BF16, kind="ExternalInput").ap()
    sel2_d = nc.dram_tensor("sel2", [33, 128], BF16, kind="ExternalInput").ap()
    ones2_d = nc.dram_tensor("ones2", [124, 33], BF16, kind="ExternalInput").ap()
    sela_d = nc.dram_tensor("sela", [125, 128], BF16, kind="ExternalInput").ap()
    hds_d = nc.dram_tensor("hds", [F, 2 * C * OUT], BF16, kind="ExternalInput").ap()
    y = nc.dram_tensor("y", [BL, OUT], F32, kind="ExternalOutput").ap()

    with tile.TileContext(nc) as tc:
        _emit(tc, y, xg, xt, waug2_d, cst_d, sel2_d, ones2_d, sela_d, hds_d)
    nc.compile()
    return nc


def _emit(tc, y, xg, xt, waug2_d, cst_d, sel2_d, ones2_d, sela_d, hds_d):
    nc = tc.nc
    from contextlib import ExitStack
    ctx = ExitStack()
    with ctx:
        cpool = ctx.enter_context(tc.tile_pool(name="cpool", bufs=1))
        xnpool = ctx.enter_context(tc.tile_pool(name="xnpool", bufs=8))
        tpool = ctx.enter_context(tc.tile_pool(name="tpool", bufs=4))
        spool = ctx.enter_context(tc.tile_pool(name="spool", bufs=5))
        gpool = ctx.enter_context(tc.tile_pool(name="gpool", bufs=5))
        fpool = ctx.enter_context(tc.tile_pool(name="fpool", bufs=2))
        lpsum = ctx.enter_context(tc.tile_pool(name="lpsum", bufs=2, space="PSUM"))
        fpsum = ctx.enter_context(tc.tile_pool(name="fpsum", bufs=2, space="PSUM"))
        finp = ctx.enter_context(tc.tile_pool(name="finp", bufs=2, space="PSUM"))

        # Pre-load the ONE activation table set covering exp+ln+square+copy
        # (natural_log_exp_and_others). Without this, the table pass picks
        # per-function first-match sets and the kernel pays ~2.7us table
        # reloads at every exp<->ln transition (16 of them, ~43us).
        from concourse.hw_specs import get_activation_tables
        _tabs = list(get_activation_tables(nc.m.arch).items())
        _need = {mybir.ActivationFunctionType.Exp, mybir.ActivationFunctionType.Ln,
                 mybir.ActivationFunctionType.Square, mybir.ActivationFunctionType.Copy}
        _sid = next(i for i, (_n, _f) in enumerate(_tabs) if _need <= _f)
        _ld = mybir.InstLoadActFuncSet(
            name=nc.get_next_instruction_name(), ins=[], outs=[],
            act_func_set_id=_sid)
        _ld.engine = mybir.EngineType.Activation
        nc.scalar.add_instruction(_ld)

        # ---- constants ----
        waug2 = cpool.tile([128, 2 * C], BF16)
        nc.scalar.dma_start(out=waug2[:], in_=waug2_d[:])
        cst = cpool.tile([128, 3 * C], BF16)
        nc.scalar.dma_start(out=cst[:], in_=cst_d[:])
        hds = cpool.tile([F, 2 * C * OUT], BF16)
        nc.scalar.dma_start(out=hds[:], in_=hds_d[:])
        cstA = cst[:, 0 * C:1 * C]        # rows 0:60 = 1/cw, 64:124 = 1/cw^2
        cstB = cst[:, 1 * C:2 * C]        # rows 0:60 = w2/cw, 64:124 = 1 - w2^2/cw^2
        cstC = cst[0:F, 2 * C:3 * C]      # rows 0:60 = 2*w2/cw^2

        ones2 = cpool.tile([124, 33], BF16)   # lhsT: col0 sums rows 0:60,
        nc.scalar.dma_start(out=ones2[:], in_=ones2_d[:])  # col32 rows 64:124
        sel2 = cpool.tile([33, 128], BF16)    # lhsT: row-block select for norms
        nc.scalar.dma_start(out=sel2[:], in_=sel2_d[:])
        sela = cpool.tile([125, 128], BF16)   # lhsT: broadcast stage row 124
        nc.scalar.dma_start(out=sela[:], in_=sela_d[:])
        eps1 = cpool.tile([1, 1], F32)        # l2-normalize epsilon
        nc.vector.memset(eps1[:], EPS)

        def cb(ap, p):  # broadcast a [p, C] const across FGB batches
            return ap.unsqueeze(1).broadcast_to([p, FGB, C])

        gran = {}      # g -> (xnt, xtt)
        fps = {}       # g -> fv psum tile
        sbst = {}      # g -> lp (both sbs)
        acts = {}      # g -> actt (both sbs)
        stages = {}    # fg -> stage tile
        fin = {}       # fg -> dict of finishing tiles
        NGR = NSB // 2  # 16 granules
        W2C = NW * 2 * C           # 320 cols per sb

        def issue_dma(g):
            xnt = xnpool.tile([128, 2 * SR], F8, tag="xnt", name="xnt")
            xtt = tpool.tile([128, 2 * HW2], F8, tag="xtt", name="xtt")
            if SHIP_SQ:
                nc.sync.dma_start(out=xnt[:], in_=xg[g])
            else:
                xv = xnt.rearrange("p (si k q) -> p si k q", si=2, q=128)
                nc.sync.dma_start(out=xv[:, :, :, 0:64], in_=xg[g])
            nc.sync.dma_start(out=xtt[:], in_=xt[g])
            gran[g] = (xnt, xtt)

        def logits2(g):
            _, xtt = gran[g]
            lp = lpsum.tile([128, 2 * W2C], F32)
            for si in range(2):
                for w in range(NW):
                    nc.tensor.matmul(
                        lp[:, si * W2C + w * 2 * C: si * W2C + (w + 1) * 2 * C],
                        xtt[:, si * HW2 + RP * w: si * HW2 + RP * w + 128],
                        waug2[:],
                        start=True, stop=True,
                    )
            sbst[g] = lp

        def squares2(g):
            if SHIP_SQ:
                return
            xnt, _ = gran[g]
            xv = xnt.rearrange("p (si k q) -> p si k q", si=2, q=128)
            nc.scalar.activation(
                xv[:, :, 0:K_ACT, 64:125], xv[:, :, 0:K_ACT, 0:61],
                mybir.ActivationFunctionType.Square,
            )
            nc.vector.tensor_tensor(
                out=xv[:, :, K_ACT:K_ACT + K_DVE, 64:125],
                in0=xv[:, :, K_ACT:K_ACT + K_DVE, 0:61],
                in1=xv[:, :, K_ACT:K_ACT + K_DVE, 0:61], op=MULT,
            )
            nc.gpsimd.tensor_tensor(
                out=xv[:, :, K_ACT + K_DVE:NK, 64:125],
                in0=xv[:, :, K_ACT + K_DVE:NK, 0:61],
                in1=xv[:, :, K_ACT + K_DVE:NK, 0:61], op=MULT,
            )

        def softmax2(g):
            # whole granule (2 sbs) in one op per stage: fewer instructions
            # and cross-engine hops on the serial chain
            lp = sbst.pop(g)
            expt = spool.tile([RP, 2 * W2C], F32, tag="expt")
            sums = spool.tile([RP, 2 * NW * 2], F32, tag="sums")
            rin = spool.tile([RP, 2 * NW * 2], F32, tag="rin")
            # per-half chains: half 0's full chain (incl. its scale, which
            # gates the first fv matmuls) runs before any half-1 DVE ops;
            # half 1's scale goes to Pool so the halves finish in parallel
            actt = spool.tile([RP, 2 * W2C], BF16, tag="actt")
            av = actt.rearrange("p (s k e) -> p s k e", s=2, e=C)
            ev = expt.rearrange("p (s k e) -> p s k e", s=2, e=C)
            rv = rin.rearrange("p (s k) -> p s k", s=2)
            for s_ in range(2):
                nc.scalar.activation(
                    expt[:, s_ * W2C:(s_ + 1) * W2C],
                    lp[0:RP, s_ * W2C:(s_ + 1) * W2C],
                    mybir.ActivationFunctionType.Exp,
                )
                nc.vector.reduce_sum(
                    out=sums[:, s_ * NW * 2:(s_ + 1) * NW * 2],
                    in_=expt[:, s_ * W2C:(s_ + 1) * W2C]
                        .rearrange("p (k e) -> p k e", e=C),
                    axis=mybir.AxisListType.X,
                )
                nc.vector.reciprocal(rin[:, s_ * NW * 2:(s_ + 1) * NW * 2],
                                     sums[:, s_ * NW * 2:(s_ + 1) * NW * 2])
                eng = nc.vector if s_ == 0 else nc.gpsimd
                eng.tensor_tensor(
                    out=av[:, s_], in0=ev[:, s_],
                    in1=rv[:, s_].unsqueeze(2).broadcast_to([RP, NW * 2, C]),
                    op=MULT,
                )
            acts[g] = actt

        def fv2g(g):
            xnt, _ = gran.pop(g)
            actt = acts.pop(g)
            fp = fpsum.tile([128, 2 * SB * C], F32)
            for si in range(2):
                sb = 2 * g + si
                fg, s = divmod(sb, SBPF)
                if s == 0:
                    stages[fg] = gpool.tile([128, NG], BF16, tag="stage",
                                            name="stage")
                for b in range(SB):
                    h, wb = divmod(b, 4)
                    for c5 in range(CH):
                        w = wb * CH + c5
                        o = (si * NK + b * CH + c5) * 128
                        nc.tensor.matmul(
                            fp[:, (si * SB + b) * C:(si * SB + b + 1) * C],
                            xnt[0:RP, o:o + 128],
                            actt[:, si * W2C + (w * 2 + h) * C:
                                 si * W2C + (w * 2 + h + 1) * C],
                            start=(c5 == 0), stop=(c5 == CH - 1),
                        )
            fps[g] = fp

        def stage_copy(g):
            # deferred one iteration: fv(g) is long done, so this never
            # stalls the ACT queue ahead of the next exp
            fp = fps.pop(g)
            fg = (2 * g) // SBPF
            s = (2 * g) % SBPF
            nc.scalar.copy(stages[fg][:, s * SB * C:(s + 2) * SB * C], fp[:])

        # ---- finishing: phases contain ops for one engine layer each and
        # consume only results from previous iterations, so in-order engine
        # queues never convoy on cross-engine waits ----
        def finP0(fg):  # PE: asum broadcast matmul
            d = fin.setdefault(fg, {})
            asb = finp.tile([128, NG], F32, tag="fin")
            nc.tensor.matmul(asb[:], sela[:], stages[fg][0:125, :],
                             start=True, stop=True)
            d["asb"] = asb

        def finP1(fg):  # DVE
            d = fin[fg]
            asbs = fpool.tile([128, NG], BF16, tag="asbs")
            nc.vector.tensor_copy(asbs[:], d.pop("asb")[:])
            t1 = fpool.tile([128, NG], BF16, tag="t1")
            nc.vector.tensor_tensor(out=t1.rearrange("p (g e) -> p g e", e=C),
                                    in0=stages[fg].rearrange("p (g e) -> p g e", e=C),
                                    in1=cb(cstA, 128), op=MULT)
            d["asbs"] = asbs
            d["t1"] = t1

        def finP2(fg):  # DVE
            d = fin[fg]
            m1 = fpool.tile([128, NG], BF16, tag="m1")
            nc.vector.tensor_tensor(out=m1.rearrange("p (g e) -> p g e", e=C),
                                    in0=d.pop("asbs").rearrange("p (g e) -> p g e", e=C),
                                    in1=cb(cstB, 128), op=MULT)
            fvn = fpool.tile([128, NG], BF16, tag="fvn")
            nc.vector.tensor_sub(fvn[:], d.pop("t1")[:], m1[:])
            u4 = fpool.tile([64 + F, NG], BF16, tag="u4")
            nc.vector.tensor_tensor(
                out=u4[64:64 + F, :].rearrange("p (g e) -> p g e", e=C),
                in0=stages[fg][0:F].rearrange("p (g e) -> p g e", e=C),
                in1=cb(cstC, F), op=MULT)
            nc.vector.tensor_sub(fvn[64:64 + F, :], fvn[64:64 + F, :],
                                 u4[64:64 + F, :])
            q1 = fpool.tile([128, NG], BF16, tag="q1")
            nc.vector.tensor_mul(q1[:], fvn[:], fvn[:])
            d["fvn"] = fvn
            d["q1"] = q1

        def finP3(fg):  # PE: norm partition-sums
            d = fin[fg]
            r12 = finp.tile([33, NG], F32, tag="fin")
            nc.tensor.matmul(r12[:], ones2[:], d.pop("q1")[0:124, :],
                             start=True, stop=True)
            d["r12"] = r12

        def finP4(fg):  # ACT (row0 rsqrt) + DVE (r2c reduce)
            d = fin[fg]
            r12 = d["r12"]
            nrB = fpool.tile([33, NG], BF16, tag="nrB")
            nc.vector.memset(nrB[:], 0.0)
            # 1/sqrt(x+eps) = exp(-0.5*ln(x+eps)): stays on the exp/ln
            # activation table set (Sqrt would force a ~2.7us table swap)
            ln1 = fpool.tile([1, NG], F32, tag="ln1")
            nc.scalar.activation(ln1[:], r12[0:1, :],
                                 mybir.ActivationFunctionType.Ln, bias=eps1[:])
            nc.scalar.activation(nrB[0:1, :], ln1[:],
                                 mybir.ActivationFunctionType.Exp, scale=-0.5)
            r2c = fpool.tile([1, FGB], F32, tag="r2c")
            nc.vector.reduce_sum(out=r2c[:],
                                 in_=r12[32:33, :].rearrange("p (g e) -> p g e", e=C),
                                 axis=mybir.AxisListType.X)
            d["nrB"] = nrB
            d["r2c"] = r2c

        def finP5(fg):  # ACT (global rsqrt) + Pool (bcast into nrB row 32)
            d = fin[fg]
            ln2 = fpool.tile([1, FGB], F32, tag="ln2")
            nc.scalar.activation(ln2[:], d.pop("r2c")[:],
                                 mybir.ActivationFunctionType.Ln, bias=eps1[:])
            nr2 = fpool.tile([1, FGB], BF16, tag="nr2")
            nc.scalar.activation(nr2[:], ln2[:],
                                 mybir.ActivationFunctionType.Exp, scale=-0.5)
            d["nr2"] = nr2
            d.pop("r12")

        def finP5b(fg):  # Pool: bcast global rsqrt (nr2 from prev iteration)
            d = fin[fg]
            nc.gpsimd.tensor_copy(
                d["nrB"][32:33, :].rearrange("p (g e) -> p g e", e=C),
                d.pop("nr2").unsqueeze(2).broadcast_to([1, FGB, C]),
            )

        def finP6(fg):  # PE: norm broadcast matmul
            d = fin[fg]
            nb = finp.tile([128, NG], F32, tag="fin")
            nc.tensor.matmul(nb[:], sel2[:], d.pop("nrB")[:],
                             start=True, stop=True)
            d["nb"] = nb

        def finP7(fg):  # DVE: normalize + split fv2 copy
            d = fin[fg]
            nbs = fpool.tile([128, NG], BF16, tag="nbs")
            nc.vector.tensor_copy(nbs[:], d.pop("nb")[:])
            fvnn = fpool.tile([128, NG], BF16, tag="fvnn")
            nc.vector.tensor_mul(fvnn[:], d.pop("fvn")[:], nbs[:])
            fv2c = fpool.tile([F, NG], BF16, tag="fv2c")
            nc.vector.tensor_copy(fv2c[:], fvnn[64:64 + F, :])
            d["fvnn"] = fvnn
            d["fv2c"] = fv2c

        def finP8(fg):  # PE: head matmuls
            d = fin[fg]
            stages.pop(fg)
            hp = finp.tile([FGB, OUT], F32, tag="fin")
            for ci in range(C):
                nc.tensor.matmul(
                    hp[:], d.pop("fvnn")[0:F, ci::C] if ci == C - 1 else
                    d["fvnn"][0:F, ci::C],
                    hds[:, ci * OUT:(ci + 1) * OUT],
                    start=(ci == 0), stop=False,
                )
            for ci in range(C):
                nc.tensor.matmul(
                    hp[:], d.pop("fv2c")[:, ci::C] if ci == C - 1 else
                    d["fv2c"][:, ci::C],
                    hds[:, (C + ci) * OUT:(C + ci + 1) * OUT],
                    start=False, stop=(ci == C - 1),
                )
            d["hp"] = hp

        def finP9(fg):  # ACT copy + DMA out
            d = fin.pop(fg)
            yt = fpool.tile([FGB, OUT], F32, tag="yt")
            nc.scalar.copy(yt[:], d.pop("hp")[:])
            nc.sync.dma_start(out=y[fg * FGB:(fg + 1) * FGB, :], in_=yt[:])

        FINPH = (finP0, finP1, finP2, finP3, finP4, finP5, finP5b, finP6,
                 finP7, finP8, finP9)
        # fv trails logits by DG granules; stage copy trails fv by 1; fin
        # phases trail the group's last stage copy, one phase per iteration
        DG = 4
        # PE phases (P0/P3/P6/P8) get a 2-iteration dependency lead over
        # their producing DVE/ACT phases: the PE runs ~1 iteration ahead of
        # the other engines, so 1-iteration spacing still head-blocks it.
        OFFS = (0, 1, 2, 4, 5, 6, 7, 8, 9, 11, 12)
        finsched = {}
        for fg in range(NFG):
            for ph in range(11):
                finsched.setdefault(4 * fg + 10 + OFFS[ph], []).append((ph, fg))

        for g in range(PF):
            issue_dma(g)
        for t in range(max(NGR + DG + 1, 4 * (NFG - 1) + 10 + OFFS[-1]) + 1):
            if t + PF < NGR:
                issue_dma(t + PF)
            if 1 <= t <= NGR:
                softmax2(t - 1)
            if t < NGR:
                logits2(t)
                squares2(t)
            if DG <= t < NGR + DG:
                fv2g(t - DG)
            for ph, fg in finsched.get(t, ()):
                FINPH[ph](fg)
            if DG + 1 <= t < NGR + DG + 1:
                stage_copy(t - DG - 1)


def _host_prep(reshaped_input, cluster_weights, covar_weights, cluster_biases,
               cluster_weights2, hidden1_weights):
    f8 = ml_dtypes.float8_e4m3
    bf = ml_dtypes.bfloat16
    x = np.ascontiguousarray(reshaped_input, dtype=np.float32)
    x8 = x.astype(f8)                                   # [B*M, F]

    # xg slab chunks: [x(60)|1|0x3 | x^2(60)|1|0x3] = 128B per (b, c5).
    # Per core/granule/partition p: [si(2), b(8), c5(5), 64 or 128].
    xr = (x8.reshape(NCORES, NSB, SB, CH, RP, F)
            .reshape(NCORES, NSB // 2, 2, SB, CH, RP, F)
            .transpose(0, 1, 5, 2, 3, 4, 6))  # [NC, g2, p, si, b, c5, f]
    w = 128 if SHIP_SQ else 64
    xgp = np.zeros((NCORES, NSB // 2, 128, 2, SB, CH, w), dtype=f8)
    xgp[:, :, :RP, :, :, :, :F] = xr
    xgp[:, :, :RP, :, :, :, F] = f8(1.0)
    if SHIP_SQ:
        x2 = np.square(x).astype(f8)
        x2r = (x2.reshape(NCORES, NSB, SB, CH, RP, F)
                 .reshape(NCORES, NSB // 2, 2, SB, CH, RP, F)
                 .transpose(0, 1, 5, 2, 3, 4, 6))
        xgp[:, :, :RP, :, :, :, 64:64 + F] = x2r
        xgp[:, :, :RP, :, :, :, 64 + F] = f8(1.0)
    xgp = np.ascontiguousarray(xgp.reshape(NCORES, NSB // 2, 128, 2 * SB * CH * w))

    # xt: transposed double-stacked, fp8, padded to 128 partitions
    x6 = (x8.reshape(NCORES, NSB, 2, 4 * M, F)
            .transpose(0, 1, 2, 4, 3))                  # [NC, NSB, 2, F, 2400]
    xtp = np.zeros((NCORES, NSB, 2, F + 1, HW2), dtype=f8)
    xtp[:, :, :, :F, :4 * M] = x6
    xtp[:, :, :, F, :] = f8(1.0)
    xtp2 = np.zeros((NCORES, NSB // 2, 128, 2 * HW2), dtype=f8)
    xtp2[:, :, :2 * (F + 1), :] = (
        xtp.reshape(NCORES, NSB // 2, 2, 2 * (F + 1), HW2)
           .transpose(0, 1, 3, 2, 4)
           .reshape(NCORES, NSB // 2, 2 * (F + 1), 2 * HW2))

    waug2 = np.zeros((128, 2 * C), dtype=bf)
    waug2[0:F, 0:C] = cluster_weights.astype(bf)
    waug2[F, 0:C] = cluster_biases.astype(bf)
    waug2[F + 1:2 * F + 1, C:2 * C] = cluster_weights.astype(bf)
    waug2[2 * F + 1, C:2 * C] = cluster_biases.astype(bf)

    cw = np.square(covar_weights.astype(np.float64)) + 1e-6       # [F, C]
    w2 = cluster_weights2[0].astype(np.float64)                   # [F, C]
    # S1/S2 pre-scale fvn so the norm sums stay inside the ACT Ln table's
    # valid range (~[1e-6, 1e16]; 1/cw^2 reaches 1e12 and r12 1e28 without
    # it). Exactly cancelled: nr' = rsqrt(r12*S^2) = nr/S and
    # fvnn = (fvn*S)*(nr/S), so no kernel-side correction.
    S1, S2 = 2.0 ** -8, 2.0 ** -24
    cst = np.zeros((128, 3 * C), dtype=np.float64)
    cst[0:F, 0 * C:1 * C] = S1 / cw
    cst[64:64 + F, 0 * C:1 * C] = S2 / np.square(cw)
    cst[0:F, 1 * C:2 * C] = S1 * w2 / cw
    cst[64:64 + F, 1 * C:2 * C] = S2 * (1.0 - np.square(w2) / np.square(cw))
    cst[0:F, 2 * C:3 * C] = S2 * 2.0 * w2 / np.square(cw)
    cst = cst.astype(bf)

    sel2 = np.zeros((33, 128), dtype=bf)
    sel2[0, 0:F] = bf(1.0)
    sel2[32, 64:64 + F] = bf(1.0)
    ones2 = np.zeros((124, 33), dtype=bf)
    ones2[0:F, 0] = bf(1.0)
    ones2[64:124, 32] = bf(1.0)
    sela = np.zeros((125, 128), dtype=bf)
    sela[124, :] = bf(1.0)

    h = hidden1_weights.astype(np.float64)              # [2*C*F, OUT]
    h1 = h[:C * F].reshape(F, C, OUT) / math.sqrt(C)    # fold 2nd l2n of fv1
    h2 = h[C * F:].reshape(F, C, OUT)
    hds = np.concatenate([h1, h2], axis=1).reshape(F, 2 * C * OUT)
    hds = np.ascontiguousarray(hds.astype(bf))

    in_maps = []
    for ci in range(NCORES):
        in_maps.append({
            "xg": np.ascontiguousarray(xgp[ci]),
            "xt": np.ascontiguousarray(xtp2[ci]),
            "waug2": waug2,
            "cst": cst,
            "sel2": sel2,
            "ones2": ones2,
            "sela": sela,
            "hds": hds,
        })
    return in_maps


_CACHE = {}


def _get_nc():
    if "nc" not in _CACHE:
        _CACHE["nc"] = _build_nc()
    return _CACHE["nc"]


def kernel(reshaped_input, cluster_weights, covar_weights, cluster_biases,
           cluster_weights2, hidden1_weights, **_kw):
    in_maps = _host_prep(reshaped_input, cluster_weights, covar_weights,
                         cluster_biases, cluster_weights2, hidden1_weights)
    nc = _get_nc()
    res = run_bass_kernel_spmd(nc, in_maps, list(range(NCORES)))
    ys = [res.results[ci]["y"] for ci in range(NCORES)]
    return np.ascontiguousarray(np.concatenate(ys, axis=0), dtype=np.float32)


if __name__ == "__main__":
    rng = np.random.default_rng(0)
    fake = {
        "reshaped_input": rng.standard_normal((B * M, F), dtype=np.float32),
        "cluster_weights": rng.standard_normal((F, C)).astype(np.float32) * 0.13,
        "covar_weights": rng.standard_normal((F, C)).astype(np.float32) * 0.13,
        "cluster_biases": rng.standard_normal((C,)).astype(np.float32) * 0.13,
        "cluster_weights2": rng.standard_normal((1, F, C)).astype(np.float32) * 0.13,
        "hidden1_weights": rng.standard_normal((2 * C * F, OUT)).astype(np.float32) * 0.35,
    }
    out = kernel(**fake)
    print("kernel output", out.shape, out.dtype, np.abs(out).mean())


# revision 33
# speedup vs baseline: 1.0216x; 1.0216x over previous
"""Trainium2 Bass kernel for nn_NetFV (NetFV pooling head), v2.

Strategy (pure data parallel over 8 cores, 256 batches each):
  - fp8e4m3 x in two streams (vs bf16: halves HBM traffic, faster FWL
    weight loads). Moving matmul operands (waug2, act) stay bf16 - the
    PE supports mixed fp8-stationary x bf16-moving exactly.
      xg: natural, host-slabbed per chunk as [x(60)|1|pad3|x^2(60)|1|pad3]
      = 128B so each fv lhsT is one contiguous FWL-eligible 128-col load
      and emits fv1|asum|fv2|asum in one accumulation group.
      xt: transposed double-stacked [x^T_A;1;x^T_B;1], 128 partitions.
  - granule (= 2 superbatch) processing: one exp/reduce/recip/scale
    chain instance per half, split DVE/Pool so halves run in parallel;
    fv trails logits by DG=4 granules so the serial chain never
    head-blocks the in-order PE queue.
  - finishing constants pre-scaled (2^-8 / 2^-24) to keep norm sums in
    the ACT Ln table's valid range; rsqrt = exp(-0.5*ln(x+eps)) plus one
    explicit table load keeps ACT on a single activation table set
    (Sqrt would cost ~2.7us reloads per use).
  - finishing split into 10 single-engine-layer phases, each consuming
    only prior-iteration results (no intra-iteration cross-engine waits);
    stage copies deferred one iteration for the same reason.
"""

import math
import sys

for _p in ("/opt/trn_rl_repo", "/opt/pypackages"):
    if _p not in sys.path:
        sys.path.append(_p)

import ml_dtypes
import numpy as np

import concourse.bacc as bacc
import concourse.bass as bass
import concourse.mybir as mybir
import concourse.tile as tile
from concourse.bass_utils import run_bass_kernel_spmd

F, M, C, OUT = 60, 600, 8, 18
B = 2048
NCORES = 8
BL = B // NCORES            # 256 batches per core
SB = 8                      # batches per superbatch
NSB = BL // SB              # 32 superbatches
FGB = 64                    # batches per finishing group
NFG = BL // FGB             # 4 finishing groups
SBPF = FGB // SB            # 8 superbatches per finishing group
CH = 5                      # chunks (of 120 rows) per batch
RP = M // CH                # 120 rows per chunk
NK = SB * CH                # 40 slab chunks per superbatch
SR = NK * 128               # 5120 slab bytes per sb per partition
HW2 = 2432                  # padded transposed width (2400 + 32)
NW = 20                     # logit windows per superbatch
NG = FGB * C                # 512 finishing columns
PF = 3                      # granule-pair prefetch depth
SHIP_SQ = True              # True: host ships x^2 in the slab (bigger DMA);
                            # False: ship compact x, strided deposit + on-chip x^2
K_ACT = 17                  # x^2 chunk split: ACT chunks
K_DVE = 16                  # DVE chunks (Pool gets the rest)

BF16 = mybir.dt.bfloat16
F8 = mybir.dt.float8e4
F32 = mybir.dt.float32
MULT = mybir.AluOpType.mult
EPS = 1e-12


def _build_nc():
    nc = bacc.Bacc(
        "TRN2", target_bir_lowering=False, debug=False,
        enable_asserts=False, num_devices=NCORES,
    )
    # One granule = 2 superbatches, one contiguous descriptor per
    # partition, 128 partitions on both streams -> 16 DMA engines.
    xg_w = 2 * SR if SHIP_SQ else 2 * SR // 2
    xg = nc.dram_tensor("xg", [NSB // 2, 128, xg_w], F8,
                        kind="ExternalInput").ap()
    xt = nc.dram_tensor("xt", [NSB // 2, 128, 2 * HW2], F8,
                        kind="ExternalInput").ap()
    waug2_d = nc.dram_tensor("waug2", [128, 2 * C], BF16,
                             kind="ExternalInput").ap()
    cst_d = nc.dram_tensor("cst", [128, 3 * C], BF16, kind="ExternalInput").ap()
    sel2_d = nc.dram_tensor("sel2", [33, 128], BF16, kind="ExternalInput").ap()
    ones2_d = nc.dram_tensor("ones2", [124, 33], BF16, kind="ExternalInput").ap()
    sela_d = nc.dram_tensor("sela", [125, 128], BF16, kind="ExternalInput").ap()
    hds_d = nc.dram_tensor("hds", [F, 2 * C * OUT], BF16, kind="ExternalInput").ap()
    y = nc.dram_tensor("y", [BL, OUT], F32, kind="ExternalOutput").ap()

    with tile.TileContext(nc) as tc:
        _emit(tc, y, xg, xt, waug2_d, cst_d, sel2_d, ones2_d, sela_d, hds_d)
    nc.compile()
    return nc


def _emit(tc, y, xg, xt, waug2_d, cst_d, sel2_d, ones2_d, sela_d, hds_d):
    nc = tc.nc
    from contextlib import ExitStack
    ctx = ExitStack()
    with ctx:
        cpool = ctx.enter_context(tc.tile_pool(name="cpool", bufs=1))
        xnpool = ctx.enter_context(tc.tile_pool(name="xnpool", bufs=10))
        tpool = ctx.enter_context(tc.tile_pool(name="tpool", bufs=4))
        spool = ctx.enter_context(tc.tile_pool(name="spool", bufs=7))
        gpool = ctx.enter_context(tc.tile_pool(name="gpool", bufs=5))
        fpool = ctx.enter_context(tc.tile_pool(name="fpool", bufs=2))
        lpsum = ctx.enter_context(tc.tile_pool(name="lpsum", bufs=2, space="PSUM"))
        fpsum = ctx.enter_context(tc.tile_pool(name="fpsum", bufs=2, space="PSUM"))
        finp = ctx.enter_context(tc.tile_pool(name="finp", bufs=2, space="PSUM"))

        # Pre-load the ONE activation table set covering exp+ln+square+copy
        # (natural_log_exp_and_others). Without this, the table pass picks
        # per-function first-match sets and the kernel pays ~2.7us table
        # reloads at every exp<->ln transition (16 of them, ~43us).
        from concourse.hw_specs import get_activation_tables
        _tabs = list(get_activation_tables(nc.m.arch).items())
        _need = {mybir.ActivationFunctionType.Exp, mybir.ActivationFunctionType.Ln,
                 mybir.ActivationFunctionType.Square, mybir.ActivationFunctionType.Copy}
        _sid = next(i for i, (_n, _f) in enumerate(_tabs) if _need <= _f)
        _ld = mybir.InstLoadActFuncSet(
            name=nc.get_next_instruction_name(), ins=[], outs=[],
            act_func_set_id=_sid)
        _ld.engine = mybir.EngineType.Activation
        nc.scalar.add_instruction(_ld)

        # ---- constants ----
        waug2 = cpool.tile([128, 2 * C], BF16)
        nc.scalar.dma_start(out=waug2[:], in_=waug2_d[:])
        cst = cpool.tile([128, 3 * C], BF16)
        nc.scalar.dma_start(out=cst[:], in_=cst_d[:])
        hds = cpool.tile([F, 2 * C * OUT], BF16)
        nc.scalar.dma_start(out=hds[:], in_=hds_d[:])
        cstA = cst[:, 0 * C:1 * C]        # rows 0:60 = 1/cw, 64:124 = 1/cw^2
        cstB = cst[:, 1 * C:2 * C]        # rows 0:60 = w2/cw, 64:124 = 1 - w2^2/cw^2
        cstC = cst[0:F, 2 * C:3 * C]      # rows 0:60 = 2*w2/cw^2

        ones2 = cpool.tile([124, 33], BF16)   # lhsT: col0 sums rows 0:60,
        nc.scalar.dma_start(out=ones2[:], in_=ones2_d[:])  # col32 rows 64:124
        sel2 = cpool.tile([33, 128], BF16)    # lhsT: row-block select for norms
        nc.scalar.dma_start(out=sel2[:], in_=sel2_d[:])
        sela = cpool.tile([125, 128], BF16)   # lhsT: broadcast stage row 124
        nc.scalar.dma_start(out=sela[:], in_=sela_d[:])
        eps1 = cpool.tile([1, 1], F32)        # l2-normalize epsilon
        nc.vector.memset(eps1[:], EPS)

        def cb(ap, p):  # broadcast a [p, C] const across FGB batches
            return ap.unsqueeze(1).broadcast_to([p, FGB, C])

        gran = {}      # g -> (xnt, xtt)
        fps = {}       # g -> fv psum tile
        sbst = {}      # g -> lp (both sbs)
        acts = {}      # g -> actt (both sbs)
        stages = {}    # fg -> stage tile
        fin = {}       # fg -> dict of finishing tiles
        NGR = NSB // 2  # 16 granules
        W2C = NW * 2 * C           # 320 cols per sb

        def issue_dma(g):
            xnt = xnpool.tile([128, 2 * SR], F8, tag="xnt", name="xnt")
            xtt = tpool.tile([128, 2 * HW2], F8, tag="xtt", name="xtt")
            if SHIP_SQ:
                nc.sync.dma_start(out=xnt[:], in_=xg[g])
            else:
                xv = xnt.rearrange("p (si k q) -> p si k q", si=2, q=128)
                nc.sync.dma_start(out=xv[:, :, :, 0:64], in_=xg[g])
            nc.sync.dma_start(out=xtt[:], in_=xt[g])
            gran[g] = (xnt, xtt)

        def logits2(g):
            _, xtt = gran[g]
            lp = lpsum.tile([128, 2 * W2C], F32)
            for si in range(2):
                for w in range(NW):
                    nc.tensor.matmul(
                        lp[:, si * W2C + w * 2 * C: si * W2C + (w + 1) * 2 * C],
                        xtt[:, si * HW2 + RP * w: si * HW2 + RP * w + 128],
                        waug2[:],
                        start=True, stop=True,
                    )
            sbst[g] = lp

        def squares2(g):
            if SHIP_SQ:
                return
            xnt, _ = gran[g]
            xv = xnt.rearrange("p (si k q) -> p si k q", si=2, q=128)
            nc.scalar.activation(
                xv[:, :, 0:K_ACT, 64:125], xv[:, :, 0:K_ACT, 0:61],
                mybir.ActivationFunctionType.Square,
            )
            nc.vector.tensor_tensor(
                out=xv[:, :, K_ACT:K_ACT + K_DVE, 64:125],
                in0=xv[:, :, K_ACT:K_ACT + K_DVE, 0:61],
                in1=xv[:, :, K_ACT:K_ACT + K_DVE, 0:61], op=MULT,
            )
            nc.gpsimd.tensor_tensor(
                out=xv[:, :, K_ACT + K_DVE:NK, 64:125],
                in0=xv[:, :, K_ACT + K_DVE:NK, 0:61],
                in1=xv[:, :, K_ACT + K_DVE:NK, 0:61], op=MULT,
            )

        def softmax2(g):
            # whole granule (2 sbs) in one op per stage: fewer instructions
            # and cross-engine hops on the serial chain
            lp = sbst.pop(g)
            expt = spool.tile([RP, 2 * W2C], F32, tag="expt")
            sums = spool.tile([RP, 2 * NW * 2], F32, tag="sums")
            rin = spool.tile([RP, 2 * NW * 2], F32, tag="rin")
            # per-half chains: half 0's full chain (incl. its scale, which
            # gates the first fv matmuls) runs before any half-1 DVE ops;
            # half 1's scale goes to Pool so the halves finish in parallel
            actt = spool.tile([RP, 2 * W2C], BF16, tag="actt")
            av = actt.rearrange("p (s k e) -> p s k e", s=2, e=C)
            ev = expt.rearrange("p (s k e) -> p s k e", s=2, e=C)
            rv = rin.rearrange("p (s k) -> p s k", s=2)
            for s_ in range(2):
                nc.scalar.activation(
                    expt[:, s_ * W2C:(s_ + 1) * W2C],
                    lp[0:RP, s_ * W2C:(s_ + 1) * W2C],
                    mybir.ActivationFunctionType.Exp,
                )
                nc.vector.reduce_sum(
                    out=sums[:, s_ * NW * 2:(s_ + 1) * NW * 2],
                    in_=expt[:, s_ * W2C:(s_ + 1) * W2C]
                        .rearrange("p (k e) -> p k e", e=C),
                    axis=mybir.AxisListType.X,
                )
                nc.vector.reciprocal(rin[:, s_ * NW * 2:(s_ + 1) * NW * 2],
                                     sums[:, s_ * NW * 2:(s_ + 1) * NW * 2])
                eng = nc.vector if s_ == 0 else nc.gpsimd
                eng.tensor_tensor(
                    out=av[:, s_], in0=ev[:, s_],
                    in1=rv[:, s_].unsqueeze(2).broadcast_to([RP, NW * 2, C]),
                    op=MULT,
                )
            acts[g] = actt

        def fv2g(g):
            xnt, _ = gran.pop(g)
            actt = acts.pop(g)
            fp = fpsum.tile([128, 2 * SB * C], F32)
            for si in range(2):
                sb = 2 * g + si
                fg, s = divmod(sb, SBPF)
                if s == 0:
                    stages[fg] = gpool.tile([128, NG], BF16, tag="stage",
                                            name="stage")
                for b in range(SB):
                    h, wb = divmod(b, 4)
                    for c5 in range(CH):
                        w = wb * CH + c5
                        o = (si * NK + b * CH + c5) * 128
                        nc.tensor.matmul(
                            fp[:, (si * SB + b) * C:(si * SB + b + 1) * C],
                            xnt[0:RP, o:o + 128],
                            actt[:, si * W2C + (w * 2 + h) * C:
                                 si * W2C + (w * 2 + h + 1) * C],
                            start=(c5 == 0), stop=(c5 == CH - 1),
                        )
            fps[g] = fp

        def stage_copy(g):
            # deferred one iteration: fv(g) is long done, so this never
            # stalls the ACT queue ahead of the next exp
            fp = fps.pop(g)
            fg = (2 * g) // SBPF
            s = (2 * g) % SBPF
            nc.scalar.copy(stages[fg][:, s * SB * C:(s + 2) * SB * C], fp[:])

        # ---- finishing: phases contain ops for one engine layer each and
        # consume only results from previous iterations, so in-order engine
        # queues never convoy on cross-engine waits ----
        def finP0(fg):  # PE: asum broadcast matmul
            d = fin.setdefault(fg, {})
            asb = finp.tile([128, NG], F32, tag="fin")
            nc.tensor.matmul(asb[:], sela[:], stages[fg][0:125, :],
                             start=True, stop=True)
            d["asb"] = asb

        def finP1(fg):  # DVE
            d = fin[fg]
            asbs = fpool.tile([128, NG], BF16, tag="asbs")
            nc.vector.tensor_copy(asbs[:], d.pop("asb")[:])
            t1 = fpool.tile([128, NG], BF16, tag="t1")
            nc.vector.tensor_tensor(out=t1.rearrange("p (g e) -> p g e", e=C),
                                    in0=stages[fg].rearrange("p (g e) -> p g e", e=C),
                                    in1=cb(cstA, 128), op=MULT)
            d["asbs"] = asbs
            d["t1"] = t1

        def finP2(fg):  # DVE
            d = fin[fg]
            m1 = fpool.tile([128, NG], BF16, tag="m1")
            nc.vector.tensor_tensor(out=m1.rearrange("p (g e) -> p g e", e=C),
                                    in0=d.pop("asbs").rearrange("p (g e) -> p g e", e=C),
                                    in1=cb(cstB, 128), op=MULT)
            fvn = fpool.tile([128, NG], BF16, tag="fvn")
            nc.vector.tensor_sub(fvn[:], d.pop("t1")[:], m1[:])
            u4 = fpool.tile([64 + F, NG], BF16, tag="u4")
            nc.vector.tensor_tensor(
                out=u4[64:64 + F, :].rearrange("p (g e) -> p g e", e=C),
                in0=stages[fg][0:F].rearrange("p (g e) -> p g e", e=C),
                in1=cb(cstC, F), op=MULT)
            nc.vector.tensor_sub(fvn[64:64 + F, :], fvn[64:64 + F, :],
                                 u4[64:64 + F, :])
            q1 = fpool.tile([128, NG], BF16, tag="q1")
            nc.vector.tensor_mul(q1[:], fvn[:], fvn[:])
            d["fvn"] = fvn
            d["q1"] = q1

        def finP3(fg):  # PE: norm partition-sums
            d = fin[fg]
            r12 = finp.tile([33, NG], F32, tag="fin")
            nc.tensor.matmul(r12[:], ones2[:], d.pop("q1")[0:124, :],
                             start=True, stop=True)
            d["r12"] = r12

        def finP4(fg):  # ACT (row0 rsqrt) + DVE (r2c reduce)
            d = fin[fg]
            r12 = d["r12"]
            nrB = fpool.tile([33, NG], BF16, tag="nrB")
            nc.vector.memset(nrB[:], 0.0)
            # 1/sqrt(x+eps) = exp(-0.5*ln(x+eps)): stays on the exp/ln
            # activation table set (Sqrt would force a ~2.7us table swap)
            ln1 = fpool.tile([1, NG], F32, tag="ln1")
            nc.scalar.activation(ln1[:], r12[0:1, :],
                                 mybir.ActivationFunctionType.Ln, bias=eps1[:])
            nc.scalar.activation(nrB[0:1, :], ln1[:],
                                 mybir.ActivationFunctionType.Exp, scale=-0.5)
            r2c = fpool.tile([1, FGB], F32, tag="r2c")
            nc.vector.reduce_sum(out=r2c[:],
                                 in_=r12[32:33, :].rearrange("p (g e) -> p g e", e=C),
                                 axis=mybir.AxisListType.X)
            d["nrB"] = nrB
            d["r2c"] = r2c

        def finP5(fg):  # ACT (global rsqrt) + Pool (bcast into nrB row 32)
            d = fin[fg]
            ln2 = fpool.tile([1, FGB], F32, tag="ln2")
            nc.scalar.activation(ln2[:], d.pop("r2c")[:],
                                 mybir.ActivationFunctionType.Ln, bias=eps1[:])
            nr2 = fpool.tile([1, FGB], BF16, tag="nr2")
            nc.scalar.activation(nr2[:], ln2[:],
                                 mybir.ActivationFunctionType.Exp, scale=-0.5)
            d["nr2"] = nr2
            d.pop("r12")

        def finP5b(fg):  # Pool: bcast global rsqrt (nr2 from prev iteration)
            d = fin[fg]
            nc.gpsimd.tensor_copy(
                d["nrB"][32:33, :].rearrange("p (g e) -> p g e", e=C),
                d.pop("nr2").unsqueeze(2).broadcast_to([1, FGB, C]),
            )

        def finP6(fg):  # PE: norm broadcast matmul
            d = fin[fg]
            nb = finp.tile([128, NG], F32, tag="fin")
            nc.tensor.matmul(nb[:], sel2[:], d.pop("nrB")[:],
                             start=True, stop=True)
            d["nb"] = nb

        def finP7(fg):  # DVE: normalize + split fv2 copy
            d = fin[fg]
            nbs = fpool.tile([128, NG], BF16, tag="nbs")
            nc.vector.tensor_copy(nbs[:], d.pop("nb")[:])
            fvnn = fpool.tile([128, NG], BF16, tag="fvnn")
            nc.vector.tensor_mul(fvnn[:], d.pop("fvn")[:], nbs[:])
            fv2c = fpool.tile([F, NG], BF16, tag="fv2c")
            nc.vector.tensor_copy(fv2c[:], fvnn[64:64 + F, :])
            d["fvnn"] = fvnn
            d["fv2c"] = fv2c

        def finP8(fg):  # PE: head matmuls
            d = fin[fg]
            stages.pop(fg)
            hp = finp.tile([FGB, OUT], F32, tag="fin")
            for ci in range(C):
                nc.tensor.matmul(
                    hp[:], d.pop("fvnn")[0:F, ci::C] if ci == C - 1 else
                    d["fvnn"][0:F, ci::C],
                    hds[:, ci * OUT:(ci + 1) * OUT],
                    start=(ci == 0), stop=False,
                )
            for ci in range(C):
                nc.tensor.matmul(
                    hp[:], d.pop("fv2c")[:, ci::C] if ci == C - 1 else
                    d["fv2c"][:, ci::C],
                    hds[:, (C + ci) * OUT:(C + ci + 1) * OUT],
                    start=False, stop=(ci == C - 1),
                )
            d["hp"] = hp

        def finP9(fg):  # ACT copy + DMA out
            d = fin.pop(fg)
            yt = fpool.tile([FGB, OUT], F32, tag="yt")
            nc.scalar.copy(yt[:], d.pop("hp")[:])
            nc.sync.dma_start(out=y[fg * FGB:(fg + 1) * FGB, :], in_=yt[:])

        FINPH = (finP0, finP1, finP2, finP3, finP4, finP5, finP5b, finP6,
                 finP7, finP8, finP9)
        # fv trails logits by DG granules; stage copy trails fv by 1; fin
        # phases trail the group's last stage copy, one phase per iteration
        DG = 6
        # PE phases (P0/P3/P6/P8) get a 2-iteration dependency lead over
        # their producing DVE/ACT phases: the PE runs ~1 iteration ahead of
        # the other engines, so 1-iteration spacing still head-blocks it.
        OFFS = (0, 1, 2, 4, 5, 6, 7, 8, 9, 11, 12)
        finsched = {}
        for fg in range(NFG):
            for ph in range(11):
                finsched.setdefault(4 * fg + 12 + OFFS[ph], []).append((ph, fg))

        for g in range(PF):
            issue_dma(g)
        for t in range(max(NGR + DG + 1, 4 * (NFG - 1) + 12 + OFFS[-1]) + 1):
            if t + PF < NGR:
                issue_dma(t + PF)
            if 1 <= t <= NGR:
                softmax2(t - 1)
            if t < NGR:
                logits2(t)
                squares2(t)
            if DG <= t < NGR + DG:
                fv2g(t - DG)
            for ph, fg in finsched.get(t, ()):
                FINPH[ph](fg)
            if DG + 1 <= t < NGR + DG + 1:
                stage_copy(t - DG - 1)


def _host_prep(reshaped_input, cluster_weights, covar_weights, cluster_biases,
               cluster_weights2, hidden1_weights):
    f8 = ml_dtypes.float8_e4m3
    bf = ml_dtypes.bfloat16
    x = np.ascontiguousarray(reshaped_input, dtype=np.float32)
    x8 = x.astype(f8)                                   # [B*M, F]

    # xg slab chunks: [x(60)|1|0x3 | x^2(60)|1|0x3] = 128B per (b, c5).
    # Per core/granule/partition p: [si(2), b(8), c5(5), 64 or 128].
    xr = (x8.reshape(NCORES, NSB, SB, CH, RP, F)
            .reshape(NCORES, NSB // 2, 2, SB, CH, RP, F)
            .transpose(0, 1, 5, 2, 3, 4, 6))  # [NC, g2, p, si, b, c5, f]
    w = 128 if SHIP_SQ else 64
    xgp = np.zeros((NCORES, NSB // 2, 128, 2, SB, CH, w), dtype=f8)
    xgp[:, :, :RP, :, :, :, :F] = xr
    xgp[:, :, :RP, :, :, :, F] = f8(1.0)
    if SHIP_SQ:
        x2 = np.square(x).astype(f8)
        x2r = (x2.reshape(NCORES, NSB, SB, CH, RP, F)
                 .reshape(NCORES, NSB // 2, 2, SB, CH, RP, F)
                 .transpose(0, 1, 5, 2, 3, 4, 6))
        xgp[:, :, :RP, :, :, :, 64:64 + F] = x2r
        xgp[:, :, :RP, :, :, :, 64 + F] = f8(1.0)
    xgp = np.ascontiguousarray(xgp.reshape(NCORES, NSB // 2, 128, 2 * SB * CH * w))

    # xt: transposed double-stacked, fp8, padded to 128 partitions
    x6 = (x8.reshape(NCORES, NSB, 2, 4 * M, F)
            .transpose(0, 1, 2, 4, 3))                  # [NC, NSB, 2, F, 2400]
    xtp = np.zeros((NCORES, NSB, 2, F + 1, HW2), dtype=f8)
    xtp[:, :, :, :F, :4 * M] = x6
    xtp[:, :, :, F, :] = f8(1.0)
    xtp2 = np.zeros((NCORES, NSB // 2, 128, 2 * HW2), dtype=f8)
    xtp2[:, :, :2 * (F + 1), :] = (
        xtp.reshape(NCORES, NSB // 2, 2, 2 * (F + 1), HW2)
           .transpose(0, 1, 3, 2, 4)
           .reshape(NCORES, NSB // 2, 2 * (F + 1), 2 * HW2))

    waug2 = np.zeros((128, 2 * C), dtype=bf)
    waug2[0:F, 0:C] = cluster_weights.astype(bf)
    waug2[F, 0:C] = cluster_biases.astype(bf)
    waug2[F + 1:2 * F + 1, C:2 * C] = cluster_weights.astype(bf)
    waug2[2 * F + 1, C:2 * C] = cluster_biases.astype(bf)

    cw = np.square(covar_weights.astype(np.float64)) + 1e-6       # [F, C]
    w2 = cluster_weights2[0].astype(np.float64)                   # [F, C]
    # S1/S2 pre-scale fvn so the norm sums stay inside the ACT Ln table's
    # valid range (~[1e-6, 1e16]; 1/cw^2 reaches 1e12 and r12 1e28 without
    # it). Exactly cancelled: nr' = rsqrt(r12*S^2) = nr/S and
    # fvnn = (fvn*S)*(nr/S), so no kernel-side correction.
    S1, S2 = 2.0 ** -8, 2.0 ** -24
    cst = np.zeros((128, 3 * C), dtype=np.float64)
    cst[0:F, 0 * C:1 * C] = S1 / cw
    cst[64:64 + F, 0 * C:1 * C] = S2 / np.square(cw)
    cst[0:F, 1 * C:2 * C] = S1 * w2 / cw
    cst[64:64 + F, 1 * C:2 * C] = S2 * (1.0 - np.square(w2) / np.square(cw))
    cst[0:F, 2 * C:3 * C] = S2 * 2.0 * w2 / np.square(cw)
    cst = cst.astype(bf)

    sel2 = np.zeros((33, 128), dtype=bf)
    sel2[0, 0:F] = bf(1.0)
    sel2[32, 64:64 + F] = bf(1.0)
    ones2 = np.zeros((124, 33), dtype=bf)
    ones2[0:F, 0] = bf(1.0)
    ones2[64:124, 32] = bf(1.0)
    sela = np.zeros((125, 128), dtype=bf)
    sela[124, :] = bf(1.0)

    h = hidden1_weights.astype(np.float64)              # [2*C*F, OUT]
    h1 = h[:C * F].reshape(F, C, OUT) / math.sqrt(C)    # fold 2nd l2n of fv1
    h2 = h[C * F:].reshape(F, C, OUT)
    hds = np.concatenate([h1, h2], axis=1).reshape(F, 2 * C * OUT)
    hds = np.ascontiguousarray(hds.astype(bf))

    in_maps = []
    for ci in range(NCORES):
        in_maps.append({
            "xg": np.ascontiguousarray(xgp[ci]),
            "xt": np.ascontiguousarray(xtp2[ci]),
            "waug2": waug2,
            "cst": cst,
            "sel2": sel2,
            "ones2": ones2,
            "sela": sela,
            "hds": hds,
        })
    return in_maps


_CACHE = {}


def _get_nc():
    if "nc" not in _CACHE:
        _CACHE["nc"] = _build_nc()
    return _CACHE["nc"]


def kernel(reshaped_input, cluster_weights, covar_weights, cluster_biases,
           cluster_weights2, hidden1_weights, **_kw):
    in_maps = _host_prep(reshaped_input, cluster_weights, covar_weights,
                         cluster_biases, cluster_weights2, hidden1_weights)
    nc = _get_nc()
    res = run_bass_kernel_spmd(nc, in_maps, list(range(NCORES)))
    ys = [res.results[ci]["y"] for ci in range(NCORES)]
    return np.ascontiguousarray(np.concatenate(ys, axis=0), dtype=np.float32)


if __name__ == "__main__":
    rng = np.random.default_rng(0)
    fake = {
        "reshaped_input": rng.standard_normal((B * M, F), dtype=np.float32),
        "cluster_weights": rng.standard_normal((F, C)).astype(np.float32) * 0.13,
        "covar_weights": rng.standard_normal((F, C)).astype(np.float32) * 0.13,
        "cluster_biases": rng.standard_normal((C,)).astype(np.float32) * 0.13,
        "cluster_weights2": rng.standard_normal((1, F, C)).astype(np.float32) * 0.13,
        "hidden1_weights": rng.standard_normal((2 * C * F, OUT)).astype(np.float32) * 0.35,
    }
    out = kernel(**fake)
    print("kernel output", out.shape, out.dtype, np.abs(out).mean())
